# revision 10
# baseline (speedup 1.0000x reference)
"""Trainium2 Bass kernel for nn_Decoder (overlap-add synthesis decoder).

Computes, for c: [B=8, N=512, K=32768] fp32 and V: [L=64, N=512] fp32:
    frames[b, k, l] = sum_n V[l, n] * c[b, n, k]
    y[b, t] = sum_{k,l: 32*k+l == t} frames[b, k, l]      (HOP=32, T=(K-1)*32+64)
returning y as [B, 1, T].

Distribution: data-parallel over the batch — core i computes batch element i.
V^T (tiny) is precomputed on the host and replicated to every core.

Per-core pipeline (per 512-column k-tile):
  1. DMA c[b] in [128, k_super] fp32 chunks (4 n-chunks of 128 partitions).
  2. 4 accumulating fp32 matmuls produce F = V @ c_tile in PSUM
     [64(l), 512(k)].
  3. Copy F into SBUF [64, 514]: the l<32 half at column offset 1, the
     l>=32 half at offset 2; slot [32:64, 1] carries the last frame of
     the previous tile (the overlap-add needs frame k-1).
  4. For each 128-column window j: ONE regular matmul with the stacked
     identity E = [I32; I32] ([64, 32]) computes
       out[k, j'] = F[j', k] + F[j'+32, k-1]
     i.e. transpose + overlap-add fused in a single standard matmul.
  5. Copy the PSUM result to SBUF, DMA to y (rows are k-blocks of 32
     consecutive samples -> the DRAM destination is contiguous per tile).
"""

import numpy as np

import concourse.bacc as bacc
import concourse.mybir as mybir
import concourse.tile as tile
from concourse.bass_utils import run_bass_kernel_spmd

HOP = 32
L = 64
N = 512
B = 8
K_FULL = 32768
F32 = mybir.dt.float32


def _super_plan(K, kt, k_super):
    """Split K into DMA supertiles; start small so compute ramps quickly."""
    plan = []
    pos = 0
    for size in (kt, k_super - kt):
        if size > 0 and pos + size <= K:
            plan.append((pos, size))
            pos += size
    while pos < K:
        size = min(k_super, K - pos)
        plan.append((pos, size))
        pos += size
    return plan


def build_nc(K=K_FULL, k_super=2048, kt=512, num_devices=B):
    T_out = (K - 1) * HOP + L
    nchunks = N // 128

    nc = bacc.Bacc(
        "TRN2",
        target_bir_lowering=False,
        debug=False,
        enable_asserts=False,
        num_devices=num_devices,
    )
    c_d = nc.dram_tensor("c", [N, K], F32, kind="ExternalInput").ap()
    vt_d = nc.dram_tensor("VT", [N, L], F32, kind="ExternalInput").ap()
    # host-prepared [64, 32] = I32 stacked twice along partitions
    eye_d = nc.dram_tensor("EYE2", [L, 32], F32, kind="ExternalInput").ap()
    y_d = nc.dram_tensor("y", [T_out], F32, kind="ExternalOutput").ap()

    with tile.TileContext(nc) as tc:
        with (
            tc.tile_pool(name="const", bufs=1) as const_pool,
            tc.tile_pool(name="cin", bufs=2 * nchunks) as cin_pool,
            tc.tile_pool(name="fsb", bufs=4) as f_pool,
            tc.tile_pool(name="osb", bufs=4) as o_pool,
            tc.tile_pool(name="psf", bufs=3, space="PSUM") as psf_pool,
            tc.tile_pool(name="pst", bufs=3, space="PSUM") as pst_pool,
        ):
            eye2 = const_pool.tile([L, 32], F32)
            nc.sync.dma_start(out=eye2, in_=eye_d)

            vt = []
            for i in range(nchunks):
                t = const_pool.tile([128, L], F32, name=f"vt{i}")
                nc.sync.dma_start(out=t, in_=vt_d[i * 128 : (i + 1) * 128, :])
                vt.append(t)

            prev_F = None
            t_idx = 0
            for k0, k_size in _super_plan(K, kt, k_super):
                cin = []
                for i in range(nchunks):
                    t = cin_pool.tile([128, k_size], F32, tag="cin")
                    nc.sync.dma_start(
                        out=t, in_=c_d[i * 128 : (i + 1) * 128, k0 : k0 + k_size]
                    )
                    cin.append(t)

                for cc in range(0, k_size, kt):
                    psum_F = psf_pool.tile([L, kt], F32, tag="psf")
                    for i in range(nchunks):
                        nc.tensor.matmul(
                            psum_F,
                            vt[i],
                            cin[i][:, cc : cc + kt],
                            start=(i == 0),
                            stop=(i == nchunks - 1),
                        )

                    F_sb = f_pool.tile([L, kt + 2], F32, tag="F")
                    if prev_F is None:
                        nc.vector.memset(F_sb[32:64, 1:2], 0.0)
                    else:
                        nc.vector.tensor_copy(
                            out=F_sb[32:64, 1:2], in_=prev_F[32:64, kt + 1 : kt + 2]
                        )
                    nc.vector.tensor_copy(
                        out=F_sb[0:32, 1 : kt + 1], in_=psum_F[0:32, :]
                    )
                    nc.scalar.copy(
                        out=F_sb[32:64, 2 : kt + 2], in_=psum_F[32:64, :]
                    )

                    psum_T = pst_pool.tile([128, kt // 4], F32, tag="pst")
                    for j in range(kt // 128):
                        # out[k_local, j'] = F[j', k] + F[j'+32, k-1]
                        nc.tensor.matmul(
                            psum_T[:, 32 * j : 32 * j + 32],
                            F_sb[0:64, 1 + 128 * j : 129 + 128 * j],
                            eye2,
                            start=True,
                            stop=True,
                        )

                    out_sb = o_pool.tile([128, kt // 4], F32, tag="out")
                    nc.vector.tensor_copy(out=out_sb, in_=psum_T)

                    ngrp = kt // 128
                    y_view = y_d[
                        t_idx * kt * HOP : (t_idx + 1) * kt * HOP
                    ].rearrange("(j i s) -> i j s", j=ngrp, i=128, s=HOP)
                    nc.sync.dma_start(
                        out=y_view,
                        in_=out_sb.rearrange("i (j s) -> i j s", j=ngrp),
                    )
                    prev_F = F_sb
                    t_idx += 1

            # tail: y[K*HOP : K*HOP+32] = frames[K-1, 32:64]
            psum_tail = pst_pool.tile([1, 32], F32, tag="ptail", bufs=1)
            nc.tensor.matmul(
                psum_tail,
                prev_F[32:64, kt + 1 : kt + 2],
                eye2[32:64, :],
                start=True,
                stop=True,
            )
            tail_sb = o_pool.tile([1, 32], F32, tag="tail")
            nc.vector.tensor_copy(out=tail_sb, in_=psum_tail)
            nc.sync.dma_start(out=y_d[K * HOP : K * HOP + 32], in_=tail_sb)

    nc.compile()
    return nc


_NC_CACHE = {}


def kernel(c: np.ndarray, V: np.ndarray, _trace: bool = False):
    assert c.shape == (B, N, K_FULL) and V.shape == (L, N)
    if "nc" not in _NC_CACHE:
        _NC_CACHE["nc"] = build_nc()
    nc = _NC_CACHE["nc"]

    vt = np.ascontiguousarray(V.T.astype(np.float32))
    eye2 = np.ascontiguousarray(np.tile(np.eye(32, dtype=np.float32), (2, 1)))
    in_maps = [
        {"c": np.ascontiguousarray(c[i], dtype=np.float32), "VT": vt, "EYE2": eye2}
        for i in range(B)
    ]
    res = run_bass_kernel_spmd(nc, in_maps, core_ids=list(range(B)), trace=_trace)
    y = np.stack([res.results[i]["y"] for i in range(B)])
    out = y[:, None, :].astype(np.float32)
    if _trace:
        _NC_CACHE["last_result"] = res
    return out
